# revision 1
# baseline (speedup 1.0000x reference)
"""GCN layer (segment-sum message passing + linear + graph-norm + LayerNorm
+ ReLU) on 8 Trainium2 NeuronCores, written in Bass/Tile.

Contract: kernel(**inputs) takes the FULL unsharded inputs (as produced by
setup_inputs(): feature [50000,128] f32, snorm_n [50000,1] f32,
W [128,128] f32, ln_scale/ln_bias [128] f32, src/dst [800000] int64) and
returns the full [50000,128] float32 output.

Design (dst-sharded SPMD across 8 cores):
  - core c owns dst nodes [c*6250, (c+1)*6250), in 49 tiles of 128.
  - edges are grouped per (core, dst-tile, src-half) on the host and padded
    to per-tile chunk counts (max over cores, so the single SPMD program is
    valid for every core); each chunk is 128 edges.
  - the feature table (bf16) stays in HBM; each chunk's 128 source rows are
    fetched with gpsimd.dma_gather (int16 indices => the table is addressed
    as two halves), spread over 4 SWDGE queues to overlap the per-descriptor
    HBM latency (the kernel is descriptor-latency bound).
  - segment-sum via one-hot matmul: S[e, seg] = (dstloc[e] == iota[seg]),
    with S built on-device (ACT materializes the seg-ids broadcast, DVE
    compares against a static iota pattern at bf16 2x rate); the per-tile
    h [seg, feat] accumulates over chunks in PSUM (f32).
  - per tile: h -> bf16 -> PE transpose -> hT, then one matmul with W^T
    gives y [seg, feat] in PSUM.
  - graph-norm folds into LayerNorm scalars (LN is scale-invariant except
    for the epsilon term): rstd' = 1/sqrt(snorm^2*var + eps), and the final
    normalize+ReLU is a single ACT op (func=Relu, scale=snorm*rstd',
    bias=-mu*snorm*rstd') reading PSUM; per-tile DMA to the output.
"""
import os

import numpy as np
import ml_dtypes
from contextlib import ExitStack

from concourse import bacc, bass, mybir
import concourse.tile as tile
from concourse._compat import with_exitstack
from concourse.bass_utils import run_bass_kernel_spmd

P = 128
LN_EPS = 1e-5

LAST_EXEC_NS = None
LAST_TRACE = None
_CACHE = {}


# ---------------------------------------------------------------------------
# workaround: this walrus build accepts only ONE sync-wait condition per
# instruction; hoist extra waits into InstNoOp's on the same engine.
def _split_wide_waits(nc):
    for fn in nc.m.functions:
        for bb in fn.blocks:
            out = []
            changed = False
            for inst in list(bb.instructions):
                si = inst.sync_info
                waits = list(si.on_wait) if si and si.on_wait else []
                if len(waits) > 1:
                    changed = True
                    for wv in waits[:-1]:
                        out.append(mybir.InstNoOp(
                            name=nc.get_next_instruction_name(),
                            engine=inst.engine,
                            sync_info=mybir.SyncInfo(on_wait=[wv],
                                                     on_update=[]),
                            bass_nofuse=True))
                    inst.sync_info = mybir.SyncInfo(
                        on_wait=waits[-1:],
                        on_update=list(si.on_update) if si.on_update else [])
                out.append(inst)
            if changed:
                bb.instructions = out


def _pack_idx16_blocks(calls):
    blocks = []
    for idx in calls:
        S = len(idx) // 16
        a = idx.astype(np.int16).reshape(S, 16).T
        blocks.append(np.tile(a, (8, 1)))
    return np.concatenate(blocks, axis=1)


def _host_prep(feature, snorm_n, W, ln_scale, ln_bias, src, dst,
               NC=8, TB=3, HALF=32768):
    N, D = feature.shape
    E = src.shape[0]
    NPC = N // NC
    T = -(-NPC // P)
    src = np.asarray(src).astype(np.int64)
    dst = np.asarray(dst).astype(np.int64)

    ROWS = ((N + 127) // 128) * 128 + 128
    tab = np.zeros((ROWS, D), dtype=ml_dtypes.bfloat16)
    tab[:N] = np.asarray(feature, dtype=np.float32)

    core_of = dst // NPC
    tloc = (dst % NPC) // P
    seg = (dst % NPC) % P
    is_b = src >= HALF

    counts = np.zeros((NC, T, 2), dtype=np.int64)
    np.add.at(counts, (core_of, tloc, is_b.astype(np.int64)), 1)
    CAt = np.maximum(-(-counts[:, :, 0].max(axis=0) // P), 1).astype(int)
    CBt = np.maximum(-(-counts[:, :, 1].max(axis=0) // P), 1).astype(int)
    offA = np.concatenate(([0], np.cumsum(CAt)))
    offB = np.concatenate(([0], np.cumsum(CBt)))
    TOTA, TOTB = int(offA[-1]), int(offB[-1])

    bsizes = []
    rem = T
    while rem > 0:
        if rem <= 3:
            bsizes += [1] * rem
            rem = 0
        else:
            bsizes.append(min(TB, rem - 2))
            rem -= bsizes[-1]
    bstarts = np.concatenate(([0], np.cumsum(bsizes))).tolist()
    NB = len(bsizes)
    offAll = [0]
    for b in range(NB):
        t0, t1 = bstarts[b], bstarts[b + 1]
        offAll.append(offAll[-1] + (offA[t1] - offA[t0])
                      + (offB[t1] - offB[t0]))
    TOTALL = int(offAll[-1])
    MAXW = int(max(offAll[b + 1] - offAll[b] for b in range(NB)))

    order = np.lexsort((is_b, tloc, core_of))
    o_core, o_tloc, o_isb = core_of[order], tloc[order], is_b[order]
    o_src, o_seg = src[order], seg[order]

    dims = dict(N=N, E=E, D=D, NC=NC, NPC=NPC, T=T, TB=TB, NB=NB,
                bstarts=bstarts,
                CAt=CAt.tolist(), CBt=CBt.tolist(),
                offA=offA.tolist(), offB=offB.tolist(),
                offAll=[int(x) for x in offAll],
                TOTA=TOTA, TOTB=TOTB, TOTALL=TOTALL,
                MAXW=MAXW, HALF=HALF, ROWS=ROWS,
                ln_identity=bool(np.allclose(ln_scale, 1.0)
                                 and np.allclose(ln_bias, 0.0)))

    WT = np.ascontiguousarray(np.asarray(W, np.float32).T
                              ).astype(ml_dtypes.bfloat16)
    snorm = np.asarray(snorm_n, np.float32).reshape(-1)
    iota_full = np.tile(np.arange(P, dtype=np.float32), (P, MAXW)
                        ).astype(ml_dtypes.bfloat16)
    ident = np.eye(P, dtype=np.float32).astype(ml_dtypes.bfloat16)

    in_maps = []
    for c in range(NC):
        m = (o_core == c)
        c_tloc, c_isb = o_tloc[m], o_isb[m]
        c_src, c_seg = o_src[m], o_seg[m]

        dstAll = np.full((P, TOTALL), -1.0, dtype=np.float32)
        padA = [None] * T
        padB = [None] * T
        segA = [None] * T
        segB = [None] * T
        for t in range(T):
            for half, (C_, pad_, sg_) in enumerate(
                    ((CAt, padA, segA), (CBt, padB, segB))):
                Ct = int(C_[t])
                sel = (c_tloc == t) & (c_isb == bool(half))
                s_ = c_src[sel] - (HALF if half else 0)
                g_ = c_seg[sel]
                n = len(s_)
                assert n <= Ct * P, (c, t, half, n, Ct * P)
                so = np.argsort(s_, kind="stable")
                s_, g_ = s_[so], g_[so]
                buf = np.zeros(Ct * P, dtype=np.int64)
                buf[:n] = s_
                pad_[t] = buf
                cols = np.full(Ct * P, -1.0, dtype=np.float32)
                cols[:n] = g_
                sg_[t] = cols.reshape(Ct, P).T
        idxA_calls, idxB_calls = [], []
        for b in range(NB):
            t0, t1 = bstarts[b], bstarts[b + 1]
            idxA_calls.append(np.concatenate(padA[t0:t1]))
            idxB_calls.append(np.concatenate(padB[t0:t1]))
            blkA = np.concatenate(segA[t0:t1], axis=1)
            blkB = np.concatenate(segB[t0:t1], axis=1)
            dstAll[:, offAll[b]:offAll[b + 1]] = np.concatenate(
                (blkA, blkB), axis=1)

        snorm_tp = np.zeros((T, P), dtype=np.float32)
        v = snorm[c * NPC:(c + 1) * NPC]
        snorm_tp.reshape(-1)[:len(v)] = v
        snorm_t = np.ascontiguousarray(snorm_tp.T)

        im = {
            "table": tab,
            "idxA": _pack_idx16_blocks(idxA_calls),
            "idxB": _pack_idx16_blocks(idxB_calls),
            "dstAll": dstAll.astype(ml_dtypes.bfloat16),
            "iotaf": iota_full,
            "ident": ident,
            "wt": WT,
            "snorm": snorm_t,
        }
        if not dims["ln_identity"]:
            im["lnsc"] = np.tile(np.asarray(ln_scale, np.float32), (P, 1))
            im["lnbi"] = np.tile(np.asarray(ln_bias, np.float32), (P, 1))
        in_maps.append(im)
    return dims, in_maps


def _build(dims):
    D, T, NB = dims["D"], dims["T"], dims["NB"]
    CAt, CBt = dims["CAt"], dims["CBt"]
    offA, offB = dims["offA"], dims["offB"]
    offAll, MAXW = dims["offAll"], dims["MAXW"]
    bstarts = dims["bstarts"]
    TOTA, TOTB, TOTALL = dims["TOTA"], dims["TOTB"], dims["TOTALL"]
    HALF, ROWS = dims["HALF"], dims["ROWS"]
    bf16, f32, i16 = mybir.dt.bfloat16, mybir.dt.float32, mybir.dt.int16
    MAXA = max(offA[bstarts[b + 1]] - offA[bstarts[b]] for b in range(NB))
    MAXB = max(offB[bstarts[b + 1]] - offB[bstarts[b]] for b in range(NB))

    nc = bacc.Bacc(None, target_bir_lowering=False, num_swdge_queues=4)
    table = nc.declare_dram_parameter("table", [ROWS, D], bf16,
                                      isOutput=False)
    idxA = nc.declare_dram_parameter("idxA", [P, TOTA * 8], i16,
                                     isOutput=False)
    idxB = nc.declare_dram_parameter("idxB", [P, TOTB * 8], i16,
                                     isOutput=False)
    dstAll = nc.declare_dram_parameter("dstAll", [P, TOTALL], bf16,
                                       isOutput=False)
    iotaf = nc.declare_dram_parameter("iotaf", [P, MAXW * P], bf16,
                                      isOutput=False)
    ident = nc.declare_dram_parameter("ident", [P, P], bf16, isOutput=False)
    wt = nc.declare_dram_parameter("wt", [P, P], bf16, isOutput=False)
    snorm = nc.declare_dram_parameter("snorm", [P, T], f32, isOutput=False)
    if not dims["ln_identity"]:
        lnsc = nc.declare_dram_parameter("lnsc", [P, D], f32, isOutput=False)
        lnbi = nc.declare_dram_parameter("lnbi", [P, D], f32, isOutput=False)
    out = nc.declare_dram_parameter("out", [T * P, D], f32, isOutput=True)

    @with_exitstack
    def kern(ctx: ExitStack, tc: tile.TileContext):
        nc = tc.nc
        consts = ctx.enter_context(tc.tile_pool(name="consts", bufs=1))
        msgsA = ctx.enter_context(tc.tile_pool(name="msgsA", bufs=4))
        msgsB = ctx.enter_context(tc.tile_pool(name="msgsB", bufs=4))
        idxp = ctx.enter_context(tc.tile_pool(name="idxp", bufs=4))
        dexpp = ctx.enter_context(tc.tile_pool(name="dexp", bufs=2))
        sp = ctx.enter_context(tc.tile_pool(name="sp", bufs=2))
        hp = ctx.enter_context(tc.tile_pool(name="hp", bufs=4))
        psum = ctx.enter_context(tc.tile_pool(name="psum", bufs=2,
                                              space="PSUM"))
        red = ctx.enter_context(tc.tile_pool(name="red", bufs=4))

        iotaf_sb = consts.tile([P, MAXW * P], bf16)
        nc.sync.dma_start(iotaf_sb[:], iotaf[:])
        ident_sb = consts.tile([P, P], bf16)
        nc.sync.dma_start(ident_sb[:], ident[:])
        wt_sb = consts.tile([P, P], bf16)
        nc.sync.dma_start(wt_sb[:], wt[:])
        snorm_sb = consts.tile([P, T], f32)
        nc.sync.dma_start(snorm_sb[:], snorm[:])
        dstAll_sb = consts.tile([P, TOTALL], bf16)
        nc.sync.dma_start(dstAll_sb[:], dstAll[:])
        if not dims["ln_identity"]:
            lnsc_sb = consts.tile([P, D], f32)
            nc.sync.dma_start(lnsc_sb[:], lnsc[:])
            lnbi_sb = consts.tile([P, D], f32)
            nc.sync.dma_start(lnbi_sb[:], lnbi[:])

        eps_sb = consts.tile([P, 1], f32)
        nc.vector.memset(eps_sb[:], LN_EPS)

        tabA = table[0:HALF, :]
        tabB = table[HALF:ROWS, :]

        for b in range(NB):
            t0 = bstarts[b]
            t1 = bstarts[b + 1]
            wA = offA[t1] - offA[t0]
            wB = offB[t1] - offB[t0]
            w = wA + wB

            iA = idxp.tile([P, MAXA * 8], i16, tag="iA")
            nc.sync.dma_start(iA[:, :wA * 8],
                              idxA[:, offA[t0] * 8:offA[t1] * 8])
            iB = idxp.tile([P, MAXB * 8], i16, tag="iB")
            nc.sync.dma_start(iB[:, :wB * 8],
                              idxB[:, offB[t0] * 8:offB[t1] * 8])
            mA = msgsA.tile([P, MAXA, D], bf16, tag="msgsA")
            nc.gpsimd.dma_gather(
                out_ap=mA[:, :wA, :], in_ap=tabA,
                idxs_ap=iA[:, :wA * 8],
                num_idxs=wA * P, num_idxs_reg=wA * P, elem_size=D,
                single_packet=False, queue_num=b % 4)
            mB = msgsB.tile([P, MAXB, D], bf16, tag="msgsB")
            nc.gpsimd.dma_gather(
                out_ap=mB[:, :wB, :], in_ap=tabB,
                idxs_ap=iB[:, :wB * 8],
                num_idxs=wB * P, num_idxs_reg=wB * P, elem_size=D,
                single_packet=False, queue_num=(b + 2) % 4)

            dexp = dexpp.tile([P, MAXW, P], bf16, tag="dexp")
            nc.scalar.activation(
                dexp[:, :w, :],
                dstAll_sb[:, offAll[b]:offAll[b + 1]].unsqueeze(2)
                    .to_broadcast([P, w, P]),
                mybir.ActivationFunctionType.Copy)
            S = sp.tile([P, MAXW, P], bf16, tag="S")
            nc.vector.tensor_tensor(
                out=S[:, :w, :].rearrange("p a b -> p (a b)"),
                in0=dexp[:, :w, :].rearrange("p a b -> p (a b)"),
                in1=iotaf_sb[:, :w * P],
                op=mybir.AluOpType.is_equal)

            for t in range(t0, t1):
                CA_, CB_ = CAt[t], CBt[t]
                a0 = offA[t] - offA[t0]
                b0 = wA + (offB[t] - offB[t0])
                nchunks = CA_ + CB_
                ph = psum.tile([P, D], f32, tag="ph")
                for k in range(CA_):
                    nc.tensor.matmul(ph[:], lhsT=S[:, a0 + k, :],
                                     rhs=mA[:, a0 + k, :],
                                     start=(k == 0),
                                     stop=(k == nchunks - 1))
                for k in range(CB_):
                    nc.tensor.matmul(ph[:], lhsT=S[:, b0 + k, :],
                                     rhs=mB[:, b0 - wA + k, :],
                                     start=False,
                                     stop=(CA_ + k == nchunks - 1))
                h_sb = hp.tile([P, D], bf16, tag="h")
                nc.scalar.activation(h_sb[:], ph[:],
                                     mybir.ActivationFunctionType.Copy)
                pt = psum.tile([P, D], bf16, tag="pt")
                nc.tensor.transpose(pt[:], h_sb[:], ident_sb[:])
                ht_sb = hp.tile([P, D], bf16, tag="ht")
                nc.scalar.activation(ht_sb[:], pt[:],
                                     mybir.ActivationFunctionType.Copy)
                py = psum.tile([P, D], f32, tag="py")
                nc.tensor.matmul(py[:], lhsT=ht_sb[:], rhs=wt_sb[:],
                                 start=True, stop=True)

                st6 = red.tile([P, 6], f32, tag="st6")
                nc.vector.bn_stats(st6[:], py[:])
                agg = red.tile([P, 2], f32, tag="agg")
                nc.vector.bn_aggr(agg[:], st6[:])
                # reference applies eps AFTER graph-norm scaling:
                # rstd' = 1/sqrt(s^2 var + eps);
                # out = relu(y*(s*rstd') - mu*s*rstd')
                v2 = red.tile([P, 1], f32, tag="v2")
                nc.vector.tensor_scalar(
                    out=v2[:], in0=agg[:, 1:2],
                    scalar1=snorm_sb[:, t:t + 1],
                    scalar2=snorm_sb[:, t:t + 1],
                    op0=mybir.AluOpType.mult, op1=mybir.AluOpType.mult)
                std = red.tile([P, 1], f32, tag="std")
                nc.scalar.activation(std[:], v2[:],
                                     mybir.ActivationFunctionType.Sqrt,
                                     bias=eps_sb[:])
                rstd = red.tile([P, 1], f32, tag="rstd")
                nc.vector.reciprocal(rstd[:], std[:])
                rs = red.tile([P, 1], f32, tag="rs")
                nc.vector.tensor_scalar(
                    out=rs[:], in0=rstd[:], scalar1=snorm_sb[:, t:t + 1],
                    scalar2=None, op0=mybir.AluOpType.mult)
                bp = red.tile([P, 1], f32, tag="bp")
                nc.vector.tensor_scalar(
                    out=bp[:], in0=agg[:, 0:1], scalar1=rs[:],
                    scalar2=-1.0, op0=mybir.AluOpType.mult,
                    op1=mybir.AluOpType.mult)
                if dims["ln_identity"]:
                    y_t = hp.tile([P, D], f32, tag="y")
                    nc.scalar.activation(
                        y_t[:], py[:], mybir.ActivationFunctionType.Relu,
                        bias=bp[:], scale=rs[:])
                    nc.scalar.dma_start(out[t * P:(t + 1) * P, :], y_t[:])
                else:
                    y_t = hp.tile([P, D], f32, tag="y")
                    nc.scalar.activation(
                        y_t[:], py[:],
                        mybir.ActivationFunctionType.Identity,
                        bias=bp[:], scale=rs[:])
                    nc.vector.tensor_tensor(out=y_t[:], in0=y_t[:],
                                            in1=lnsc_sb[:],
                                            op=mybir.AluOpType.mult)
                    nc.vector.tensor_tensor(out=y_t[:], in0=y_t[:],
                                            in1=lnbi_sb[:],
                                            op=mybir.AluOpType.add)
                    yr = hp.tile([P, D], f32, tag="yr")
                    nc.scalar.activation(yr[:], y_t[:],
                                         mybir.ActivationFunctionType.Relu)
                    nc.scalar.dma_start(out[t * P:(t + 1) * P, :], yr[:])

    with tile.TileContext(nc) as tc:
        kern(tc)
    nc.compile()
    _split_wide_waits(nc)
    return nc


def kernel(feature, snorm_n, W, ln_scale, ln_bias, src, dst):
    global LAST_EXEC_NS, LAST_TRACE
    feature = np.asarray(feature, dtype=np.float32)
    snorm_n = np.asarray(snorm_n, dtype=np.float32)
    W = np.asarray(W, dtype=np.float32)
    ln_scale = np.asarray(ln_scale, dtype=np.float32)
    ln_bias = np.asarray(ln_bias, dtype=np.float32)
    src = np.asarray(src)
    dst = np.asarray(dst)

    dims, in_maps = _host_prep(feature, snorm_n, W, ln_scale, ln_bias,
                               src, dst)
    key = (dims["TOTA"], dims["TOTB"], dims["TOTALL"], dims["MAXW"],
           tuple(dims["CAt"]), tuple(dims["CBt"]), dims["ln_identity"])
    nc = _CACHE.get(key)
    if nc is None:
        nc = _build(dims)
        _CACHE[key] = nc

    trace = bool(os.environ.get("GCN_TRACE"))
    kwargs = {}
    if trace:
        kwargs = dict(trace=True, trace_cores=[0])
    br = run_bass_kernel_spmd(nc, in_maps, list(range(dims["NC"])), **kwargs)
    LAST_EXEC_NS = br.exec_time_ns
    LAST_TRACE = (br.instructions_and_trace[1]
                  if br.instructions_and_trace else None)

    NPC = dims["NPC"]
    outs = [r["out"][:NPC] for r in br.results]
    return np.concatenate(outs, axis=0)[:dims["N"]].astype(np.float32)



# revision 12
# speedup vs baseline: 1.0885x; 1.0885x over previous
"""GCN layer (segment-sum message passing + linear + graph-norm + LayerNorm
+ ReLU) on 8 Trainium2 NeuronCores, written in Bass/Tile.

Contract: kernel(**inputs) takes the FULL unsharded inputs (as produced by
setup_inputs(): feature [50000,128] f32, snorm_n [50000,1] f32,
W [128,128] f32, ln_scale/ln_bias [128] f32, src/dst [800000] int64) and
returns the full [50000,128] float32 output.

Design (dst-sharded SPMD across 8 cores), v2:
  - core c owns dst nodes [c*6250, (c+1)*6250), in 49 tiles of 128.
  - edges grouped per (core, dst-tile, src-half); each (tile, half) is ONE
    gpsimd.dma_gather call (98 total) on SWDGE queue seq%4 -> all 4 Q7 core
    pairs generate descriptors concurrently (the kernel is descriptor-rate
    bound).
  - per-call indices are padded with trailing -1 (int16): the Q7 ucode trims
    trailing negatives at runtime, so each core gathers only its REAL edge
    rows; pad slots keep stale-but-finite SBUF data (message pools are
    memset once at start) and their one-hot coefficients are 0.
  - segment-sum via one-hot matmul accumulated TRANSPOSED: per chunk,
    matmul(lhsT=m_chunk [e,f], rhs=S_chunk [e,seg]) -> hT [f,seg] in PSUM;
    S built on DVE directly via is_equal(dstAll bcast, iota bcast) (no ACT
    materialization; iota is a [128,128] const broadcast along chunks).
  - per tile: hT -> bf16 SBUF (1 ACT copy), y = matmul(lhsT=hT, rhs=W^T)
    -> [seg, f_out] PSUM. No PE transpose needed.
  - LN epilogue: rstd' = Rsqrt(snorm^2*var + eps) (1 ACT op), rs = snorm *
    rstd' (ACT), bp = -mu*rs (DVE), y = Relu(rs*y + bp) (ACT, reads PSUM),
    per-tile DMA to the output.
"""
import os

import numpy as np
import ml_dtypes
from contextlib import ExitStack

from concourse import bacc, bass, mybir
import concourse.tile as tile
from concourse._compat import with_exitstack
from concourse.bass_utils import run_bass_kernel_spmd

P = 128
LN_EPS = 1e-5

LAST_EXEC_NS = None
LAST_TRACE = None
_CACHE = {}


# ---------------------------------------------------------------------------
# workaround: this walrus build accepts only ONE sync-wait condition per
# instruction; hoist extra waits into InstNoOp's on the same engine.
def _split_wide_waits(nc):
    for fn in nc.m.functions:
        for bb in fn.blocks:
            out = []
            changed = False
            for inst in list(bb.instructions):
                si = inst.sync_info
                waits = list(si.on_wait) if si and si.on_wait else []
                if len(waits) > 1:
                    changed = True
                    for wv in waits[:-1]:
                        out.append(mybir.InstNoOp(
                            name=nc.get_next_instruction_name(),
                            engine=inst.engine,
                            sync_info=mybir.SyncInfo(on_wait=[wv],
                                                     on_update=[]),
                            bass_nofuse=True))
                    inst.sync_info = mybir.SyncInfo(
                        on_wait=waits[-1:],
                        on_update=list(si.on_update) if si.on_update else [])
                out.append(inst)
            if changed:
                bb.instructions = out


def _pack_idx16_blocks(calls):
    blocks = []
    for idx in calls:
        S = len(idx) // 16
        a = idx.astype(np.int16).reshape(S, 16).T
        blocks.append(np.tile(a, (8, 1)))
    return np.concatenate(blocks, axis=1)


def _host_prep(feature, snorm_n, W, ln_scale, ln_bias, src, dst,
               NC=8, HALF=32768, IDXB=8):
    N, D = feature.shape
    E = src.shape[0]
    NPC = N // NC
    T = -(-NPC // P)
    src = np.asarray(src).astype(np.int64)
    dst = np.asarray(dst).astype(np.int64)

    ROWS = ((N + 127) // 128) * 128 + 128
    tab = np.zeros((ROWS, D), dtype=ml_dtypes.bfloat16)
    tab[:N] = np.asarray(feature, dtype=np.float32)

    core_of = dst // NPC
    tloc = (dst % NPC) // P
    seg = (dst % NPC) % P
    is_b = src >= HALF

    counts = np.zeros((NC, T, 2), dtype=np.int64)
    np.add.at(counts, (core_of, tloc, is_b.astype(np.int64)), 1)
    # per-(tile,half) REAL gather count, equalized across cores: every core
    # pads its index list with valid dummies (row 0) up to nmax*, then -1.
    # num_idxs_reg must equal the per-core count of non-negative indices
    # (the decode-side ring accounting is derived from it), and it must be
    # an SPMD-uniform constant -> equalize.
    nmaxA = np.maximum(counts[:, :, 0].max(axis=0), 1).astype(int)
    nmaxB = np.maximum(counts[:, :, 1].max(axis=0), 1).astype(int)
    CAt = (-(-nmaxA // P)).astype(int)
    CBt = (-(-nmaxB // P)).astype(int)
    offA = np.concatenate(([0], np.cumsum(CAt)))
    offB = np.concatenate(([0], np.cumsum(CBt)))
    TOTA, TOTB = int(offA[-1]), int(offB[-1])
    Wt = (CAt + CBt).astype(int)
    offAll = np.concatenate(([0], np.cumsum(Wt)))
    TOTALL = int(offAll[-1])
    MAXW = int(Wt.max())
    MAXA, MAXB = int(CAt.max()), int(CBt.max())

    # idx DMA batches: tiles [b*IDXB, min((b+1)*IDXB, T))
    NBI = -(-T // IDXB)
    ibounds = [(b * IDXB, min((b + 1) * IDXB, T)) for b in range(NBI)]
    batchA = [int(offA[t1] - offA[t0]) for (t0, t1) in ibounds]
    batchB = [int(offB[t1] - offB[t0]) for (t0, t1) in ibounds]
    MAXBA, MAXBB = max(batchA), max(batchB)

    dims = dict(N=N, E=E, D=D, NC=NC, NPC=NPC, T=T,
                nmaxA=nmaxA.tolist(), nmaxB=nmaxB.tolist(),
                CAt=CAt.tolist(), CBt=CBt.tolist(),
                offA=offA.tolist(), offB=offB.tolist(),
                offAll=offAll.tolist(),
                TOTA=TOTA, TOTB=TOTB, TOTALL=TOTALL,
                MAXW=MAXW, MAXA=MAXA, MAXB=MAXB,
                IDXB=IDXB, NBI=NBI, MAXBA=MAXBA, MAXBB=MAXBB,
                HALF=HALF, ROWS=ROWS,
                ln_identity=bool(np.allclose(ln_scale, 1.0)
                                 and np.allclose(ln_bias, 0.0)))

    WT = np.ascontiguousarray(np.asarray(W, np.float32).T
                              ).astype(ml_dtypes.bfloat16)
    snorm = np.asarray(snorm_n, np.float32).reshape(-1)
    iota = np.tile(np.arange(P, dtype=np.float32), (P, 1)
                   ).astype(ml_dtypes.bfloat16)

    order = np.lexsort((src, is_b, tloc, core_of))
    o_core, o_tloc, o_isb = core_of[order], tloc[order], is_b[order]
    o_src, o_seg = src[order], seg[order]

    in_maps = []
    for c in range(NC):
        m = (o_core == c)
        c_tloc, c_isb = o_tloc[m], o_isb[m]
        c_src, c_seg = o_src[m], o_seg[m]

        dstAll = np.full((P, TOTALL), -1.0, dtype=np.float32)
        idxA_calls = [None] * T
        idxB_calls = [None] * T
        for t in range(T):
            for half, (C_, nm_, calls) in enumerate(
                    ((CAt, nmaxA, idxA_calls), (CBt, nmaxB, idxB_calls))):
                Ct = int(C_[t])
                nm = int(nm_[t])
                sel = (c_tloc == t) & (c_isb == bool(half))
                s_ = c_src[sel] - (HALF if half else 0)
                g_ = c_seg[sel]
                n = len(s_)
                assert n <= nm <= Ct * P, (c, t, half, n, nm, Ct * P)
                # already src-sorted from the lexsort
                buf = np.full(Ct * P, -1, dtype=np.int64)
                buf[:n] = s_
                buf[n:nm] = 0
                calls[t] = buf
                cols = np.full(Ct * P, -1.0, dtype=np.float32)
                cols[:n] = g_
                co = offAll[t] + (CAt[t] if half else 0)
                dstAll[:, co:co + Ct] = cols.reshape(Ct, P).T

        idxA = _pack_idx16_blocks(idxA_calls)
        idxB = _pack_idx16_blocks(idxB_calls)

        snorm_tp = np.zeros((T, P), dtype=np.float32)
        v = snorm[c * NPC:(c + 1) * NPC]
        snorm_tp.reshape(-1)[:len(v)] = v
        snorm_t = np.ascontiguousarray(snorm_tp.T)

        im = {
            "table": tab,
            "idxA": idxA,
            "idxB": idxB,
            "dstAll": dstAll.astype(ml_dtypes.bfloat16),
            "iota": iota,
            "wt": WT,
            "snorm": snorm_t,
            "snorm2": np.ascontiguousarray(snorm_t * snorm_t),
        }
        if not dims["ln_identity"]:
            im["lnsc"] = np.tile(np.asarray(ln_scale, np.float32), (P, 1))
            im["lnbi"] = np.tile(np.asarray(ln_bias, np.float32), (P, 1))
        in_maps.append(im)
    return dims, in_maps


def _build(dims, single_packet=False):
    D, T = dims["D"], dims["T"]
    CAt, CBt = dims["CAt"], dims["CBt"]
    NMA, NMB = dims["nmaxA"], dims["nmaxB"]
    offA, offB = dims["offA"], dims["offB"]
    offAll = dims["offAll"]
    MAXW, MAXA, MAXB = dims["MAXW"], dims["MAXA"], dims["MAXB"]
    IDXB, NBI = dims["IDXB"], dims["NBI"]
    MAXBA, MAXBB = dims["MAXBA"], dims["MAXBB"]
    TOTA, TOTB, TOTALL = dims["TOTA"], dims["TOTB"], dims["TOTALL"]
    HALF, ROWS = dims["HALF"], dims["ROWS"]
    bf16, f32, i16 = mybir.dt.bfloat16, mybir.dt.float32, mybir.dt.int16

    nc = bacc.Bacc(None, target_bir_lowering=False, num_swdge_queues=4)
    table = nc.declare_dram_parameter("table", [ROWS, D], bf16,
                                      isOutput=False)
    idxA = nc.declare_dram_parameter("idxA", [P, TOTA * 8], i16,
                                     isOutput=False)
    idxB = nc.declare_dram_parameter("idxB", [P, TOTB * 8], i16,
                                     isOutput=False)
    dstAll = nc.declare_dram_parameter("dstAll", [P, TOTALL], bf16,
                                       isOutput=False)
    iota = nc.declare_dram_parameter("iota", [P, P], bf16, isOutput=False)
    wt = nc.declare_dram_parameter("wt", [P, P], bf16, isOutput=False)
    snorm = nc.declare_dram_parameter("snorm", [P, T], f32, isOutput=False)
    snorm2 = nc.declare_dram_parameter("snorm2", [P, T], f32, isOutput=False)
    if not dims["ln_identity"]:
        lnsc = nc.declare_dram_parameter("lnsc", [P, D], f32, isOutput=False)
        lnbi = nc.declare_dram_parameter("lnbi", [P, D], f32, isOutput=False)
    out = nc.declare_dram_parameter("out", [T * P, D], f32, isOutput=True)

    MSGS_BUFS = 6
    SP_BUFS = 6

    @with_exitstack
    def kern(ctx: ExitStack, tc: tile.TileContext):
        nc = tc.nc
        consts = ctx.enter_context(tc.tile_pool(name="consts", bufs=1))
        msgsA = ctx.enter_context(tc.tile_pool(name="msgsA",
                                               bufs=MSGS_BUFS))
        msgsB = ctx.enter_context(tc.tile_pool(name="msgsB",
                                               bufs=MSGS_BUFS))
        idxpA = ctx.enter_context(tc.tile_pool(name="idxpA", bufs=3))
        idxpB = ctx.enter_context(tc.tile_pool(name="idxpB", bufs=3))
        sp = ctx.enter_context(tc.tile_pool(name="sp", bufs=SP_BUFS))
        hp = ctx.enter_context(tc.tile_pool(name="hp", bufs=4))
        psum = ctx.enter_context(tc.tile_pool(name="psum", bufs=4,
                                              space="PSUM"))
        red = ctx.enter_context(tc.tile_pool(name="red", bufs=4))

        # first idx batch load goes first so gather 0 can start ASAP
        iA_b = [None] * NBI
        iB_b = [None] * NBI
        iA_b[0] = idxpA.tile([P, MAXBA * 8], i16, tag="iA", name="iA0")
        nc.sync.dma_start(iA_b[0][:, :offA[IDXB] * 8],
                          idxA[:, :offA[IDXB] * 8])
        iB_b[0] = idxpB.tile([P, MAXBB * 8], i16, tag="iB", name="iB0")
        nc.sync.dma_start(iB_b[0][:, :offB[IDXB] * 8],
                          idxB[:, :offB[IDXB] * 8])

        iota_sb = consts.tile([P, P], bf16)
        nc.sync.dma_start(iota_sb[:], iota[:])
        wt_sb = consts.tile([P, P], bf16)
        nc.sync.dma_start(wt_sb[:], wt[:])
        snorm_sb = consts.tile([P, T], f32)
        nc.sync.dma_start(snorm_sb[:], snorm[:])
        snorm2_sb = consts.tile([P, T], f32)
        nc.sync.dma_start(snorm2_sb[:], snorm2[:])
        dstAll_sb = consts.tile([P, TOTALL], bf16)
        nc.sync.dma_start(dstAll_sb[:], dstAll[:])
        if not dims["ln_identity"]:
            lnsc_sb = consts.tile([P, D], f32)
            nc.sync.dma_start(lnsc_sb[:], lnsc[:])
            lnbi_sb = consts.tile([P, D], f32)
            nc.sync.dma_start(lnbi_sb[:], lnbi[:])

        eps_sb = consts.tile([P, 1], f32)
        nc.vector.memset(eps_sb[:], LN_EPS)

        # pre-zero all message-pool physical buffers: pad slots (whose
        # gather descriptors are skipped via trailing -1 idxs) must hold
        # finite data, since 0-coefficient x NaN would poison the matmul.
        for i in range(MSGS_BUFS):
            ta = msgsA.tile([P, MAXA, D], bf16, tag="msgsA")
            nc.vector.memset(ta[:].rearrange("p a b -> p (a b)"), 0.0)
            tb = msgsB.tile([P, MAXB, D], bf16, tag="msgsB")
            nc.vector.memset(tb[:].rearrange("p a b -> p (a b)"), 0.0)

        tabA = table[0:HALF, :]
        tabB = table[HALF:ROWS, :]

        for t in range(T):
            CA_, CB_ = CAt[t], CBt[t]
            W_ = CA_ + CB_
            bi = t // IDXB
            if t % IDXB == 0 and bi + 1 < NBI:
                nb = bi + 1
                t1 = min((nb + 1) * IDXB, T)
                iA_b[nb] = idxpA.tile([P, MAXBA * 8], i16, tag="iA",
                                      name=f"iA{nb}")
                nc.sync.dma_start(
                    iA_b[nb][:, :(offA[t1] - offA[nb * IDXB]) * 8],
                    idxA[:, offA[nb * IDXB] * 8:offA[t1] * 8])
                iB_b[nb] = idxpB.tile([P, MAXBB * 8], i16, tag="iB",
                                      name=f"iB{nb}")
                nc.sync.dma_start(
                    iB_b[nb][:, :(offB[t1] - offB[nb * IDXB]) * 8],
                    idxB[:, offB[nb * IDXB] * 8:offB[t1] * 8])
            aoff = (offA[t] - offA[bi * IDXB]) * 8
            boff = (offB[t] - offB[bi * IDXB]) * 8

            mA = msgsA.tile([P, MAXA, D], bf16, tag="msgsA")
            nc.gpsimd.dma_gather(
                out_ap=mA[:, :CA_, :], in_ap=tabA,
                idxs_ap=iA_b[bi][:, aoff:aoff + CA_ * 8],
                num_idxs=CA_ * P, num_idxs_reg=NMA[t], elem_size=D,
                single_packet=single_packet, queue_num=(2 * t) % 4)
            mB = msgsB.tile([P, MAXB, D], bf16, tag="msgsB")
            nc.gpsimd.dma_gather(
                out_ap=mB[:, :CB_, :], in_ap=tabB,
                idxs_ap=iB_b[bi][:, boff:boff + CB_ * 8],
                num_idxs=CB_ * P, num_idxs_reg=NMB[t], elem_size=D,
                single_packet=single_packet, queue_num=(2 * t + 1) % 4)

            S = sp.tile([P, MAXW, P], bf16, tag="S")
            nc.vector.tensor_tensor(
                out=S[:, :W_, :],
                in0=dstAll_sb[:, offAll[t]:offAll[t] + W_]
                    .unsqueeze(2).to_broadcast([P, W_, P]),
                in1=iota_sb[:].unsqueeze(1).to_broadcast([P, W_, P]),
                op=mybir.AluOpType.is_equal)

            phT = psum.tile([P, D], f32, tag="phT")
            for k in range(CA_):
                nc.tensor.matmul(phT[:], lhsT=mA[:, k, :],
                                 rhs=S[:, k, :],
                                 start=(k == 0),
                                 stop=(k == W_ - 1))
            for k in range(CB_):
                nc.tensor.matmul(phT[:], lhsT=mB[:, k, :],
                                 rhs=S[:, CA_ + k, :],
                                 start=False,
                                 stop=(CA_ + k == W_ - 1))
            hT_sb = hp.tile([P, D], bf16, tag="hT")
            nc.scalar.activation(hT_sb[:], phT[:],
                                 mybir.ActivationFunctionType.Copy)
            py = psum.tile([P, D], f32, tag="py")
            nc.tensor.matmul(py[:], lhsT=hT_sb[:], rhs=wt_sb[:],
                             start=True, stop=True)

            st6 = red.tile([P, 6], f32, tag="st6")
            nc.vector.bn_stats(st6[:], py[:])
            agg = red.tile([P, 2], f32, tag="agg")
            nc.vector.bn_aggr(agg[:], st6[:])
            # rstd' = 1/sqrt(snorm^2*var + eps); rs = snorm*rstd';
            # out = relu(rs*y - mu*rs)
            std = red.tile([P, 1], f32, tag="std")
            nc.scalar.activation(std[:], agg[:, 1:2],
                                 mybir.ActivationFunctionType.Sqrt,
                                 bias=eps_sb[:],
                                 scale=snorm2_sb[:, t:t + 1])
            rstd = red.tile([P, 1], f32, tag="rstd")
            nc.vector.reciprocal(rstd[:], std[:])
            rs = red.tile([P, 1], f32, tag="rs")
            nc.scalar.activation(rs[:], rstd[:],
                                 mybir.ActivationFunctionType.Identity,
                                 scale=snorm_sb[:, t:t + 1])
            bp = red.tile([P, 1], f32, tag="bp")
            nc.vector.tensor_scalar(
                out=bp[:], in0=agg[:, 0:1], scalar1=rs[:],
                scalar2=-1.0, op0=mybir.AluOpType.mult,
                op1=mybir.AluOpType.mult)
            if dims["ln_identity"]:
                y_t = hp.tile([P, D], f32, tag="y")
                nc.scalar.activation(
                    y_t[:], py[:], mybir.ActivationFunctionType.Relu,
                    bias=bp[:], scale=rs[:])
                nc.scalar.dma_start(out[t * P:(t + 1) * P, :], y_t[:])
            else:
                y_t = hp.tile([P, D], f32, tag="y")
                nc.scalar.activation(
                    y_t[:], py[:],
                    mybir.ActivationFunctionType.Identity,
                    bias=bp[:], scale=rs[:])
                nc.vector.tensor_tensor(out=y_t[:], in0=y_t[:],
                                        in1=lnsc_sb[:],
                                        op=mybir.AluOpType.mult)
                nc.vector.tensor_tensor(out=y_t[:], in0=y_t[:],
                                        in1=lnbi_sb[:],
                                        op=mybir.AluOpType.add)
                yr = hp.tile([P, D], f32, tag="yr")
                nc.scalar.activation(yr[:], y_t[:],
                                     mybir.ActivationFunctionType.Relu)
                nc.scalar.dma_start(out[t * P:(t + 1) * P, :], yr[:])

    with tile.TileContext(nc) as tc:
        kern(tc)
    nc.compile()
    _split_wide_waits(nc)
    return nc


def kernel(feature, snorm_n, W, ln_scale, ln_bias, src, dst):
    global LAST_EXEC_NS, LAST_TRACE
    feature = np.asarray(feature, dtype=np.float32)
    snorm_n = np.asarray(snorm_n, dtype=np.float32)
    W = np.asarray(W, dtype=np.float32)
    ln_scale = np.asarray(ln_scale, dtype=np.float32)
    ln_bias = np.asarray(ln_bias, dtype=np.float32)
    src = np.asarray(src)
    dst = np.asarray(dst)

    dims, in_maps = _host_prep(feature, snorm_n, W, ln_scale, ln_bias,
                               src, dst)
    key = (dims["TOTA"], dims["TOTB"], dims["TOTALL"],
           tuple(dims["CAt"]), tuple(dims["CBt"]),
           tuple(dims["nmaxA"]), tuple(dims["nmaxB"]),
           dims["ln_identity"])
    nc = _CACHE.get(key)
    if nc is None:
        nc = _build(dims)
        _CACHE[key] = nc

    trace = bool(os.environ.get("GCN_TRACE"))
    kwargs = {}
    if trace:
        kwargs = dict(trace=True, trace_cores=[0])
    br = run_bass_kernel_spmd(nc, in_maps, list(range(dims["NC"])), **kwargs)
    LAST_EXEC_NS = br.exec_time_ns
    LAST_TRACE = (br.instructions_and_trace[1]
                  if br.instructions_and_trace else None)

    NPC = dims["NPC"]
    outs = [r["out"][:NPC] for r in br.results]
    return np.concatenate(outs, axis=0)[:dims["N"]].astype(np.float32)
